# revision 8
# baseline (speedup 1.0000x reference)
"""Trainium2 Bass kernel for the plastic-network step:

    yout     = tanh(yin @ (w + alpha * hebb) + input)        [1, N]
    hebb_new = (1 - eta) * hebb + eta * outer(yin, yout)     [N, N]

Sharding: all NxN matrices (w, alpha, hebb) are split column-wise across
8 NeuronCores (512 columns each).  The matvec contraction runs over all
4096 rows of the local column shard, so every core computes its own
yout columns independently -- no collectives.  hebb is kept resident in
SBUF between the matvec pass and the hebb-update pass so it is read
from HBM exactly once.
"""

import numpy as np

import concourse.bass as bass
import concourse.bacc as bacc
import concourse.mybir as mybir
from concourse.bass_utils import run_bass_kernel_spmd
from concourse.tile import TileContext

N = 4096          # full matrix dim
NCORES = 8
S = N // NCORES   # columns per core (512)
P = 128           # SBUF partitions
T = N // P        # row tiles (32)
CHUNK = 4         # row tiles per DMA chunk -> 4*128*512*4B = 1 MiB
NCHUNKS = T // CHUNK

F32 = mybir.dt.float32

_last_results = None  # BassKernelResults of the most recent run (for test.py)


def _build() -> bacc.Bacc:
    nc = bacc.Bacc()

    w_d = nc.dram_tensor("w_sh", [N, S], F32, kind="ExternalInput")
    a_d = nc.dram_tensor("alpha_sh", [N, S], F32, kind="ExternalInput")
    h_d = nc.dram_tensor("hebb_sh", [N, S], F32, kind="ExternalInput")
    yin_d = nc.dram_tensor("yin_full", [1, N], F32, kind="ExternalInput")
    inp_d = nc.dram_tensor("input_sh", [1, S], F32, kind="ExternalInput")
    eta_d = nc.dram_tensor("eta_in", [1, 1], F32, kind="ExternalInput")

    yout_d = nc.dram_tensor("yout_sh", [1, S], F32, kind="ExternalOutput")
    hout_d = nc.dram_tensor("hebb_out", [N, S], F32, kind="ExternalOutput")

    mult = mybir.AluOpType.mult
    add = mybir.AluOpType.add

    with TileContext(nc) as tc:
        with (
            tc.tile_pool(name="const", bufs=1) as cpool,
            tc.tile_pool(name="hebb", bufs=1) as hpool,
            tc.tile_pool(name="wa", bufs=3) as wapool,
            tc.tile_pool(name="mm", bufs=2) as mpool,
            tc.tile_pool(name="ob", bufs=3) as opool,
            tc.tile_pool(name="ps", bufs=2, space="PSUM") as pspool,
            tc.tile_pool(name="psz", bufs=1, space="PSUM") as pzpool,
        ):
            # ---- small setup tensors ----
            # yin in "column" layout: yin_col[p, t] = yin[t*128 + p].
            # Both yin views pass through a DVE copy so that every matmul
            # depends only on the DVE semaphore (walrus allows a single
            # sync-wait on Matmult instructions).
            yin_colr = cpool.tile([P, T], F32, tag="yin_colr")
            nc.sync.dma_start(
                out=yin_colr[:], in_=yin_d.rearrange("a (t p) -> (a p) t", p=P)
            )
            yin_col = cpool.tile([P, T], F32, tag="yin_col")
            nc.vector.tensor_copy(out=yin_col[:], in_=yin_colr[:])
            # yin as a plain row (K=1 stationary operand for the outer product)
            yin_rowr = cpool.tile([1, N], F32, tag="yin_rowr")
            nc.sync.dma_start(out=yin_rowr[:], in_=yin_d[:])
            yin_row = cpool.tile([1, N], F32, tag="yin_row")
            nc.vector.tensor_copy(out=yin_row[:], in_=yin_rowr[:])
            inp_sb = cpool.tile([1, S], F32, tag="inp")
            nc.sync.dma_start(out=inp_sb[:], in_=inp_d[:])
            eta_sb = cpool.tile([1, 1], F32, tag="eta")
            nc.sync.dma_start(out=eta_sb[:], in_=eta_d[:])

            # Broadcast eta across partitions (0-stride DMA read from DRAM),
            # then compute (1 - eta).
            etab = cpool.tile([P, 1], F32, tag="etab")
            nc.sync.dma_start(
                out=etab.rearrange("p (c o) -> p c o", o=1),
                in_=eta_d[:].partition_broadcast(P),
            )
            ome_b = cpool.tile([P, 1], F32, tag="ome")  # 1 - eta, per partition
            nc.vector.tensor_scalar(
                out=ome_b[:], in0=etab[:], scalar1=-1.0, scalar2=1.0,
                op0=mult, op1=add,
            )

            # hebb shard stays resident in SBUF: 128 x (32*512) f32 = 8 MiB
            hebb_all = hpool.tile([P, T * S], F32, tag="hebb_all")

            # ---- phase A: z = yin @ (w + alpha*hebb) for the local columns ----
            z_ps = pzpool.tile([1, S], F32, tag="z")
            for c in range(NCHUNKS):
                r0 = c * CHUNK * P
                csl = slice(c * CHUNK * S, (c + 1) * CHUNK * S)
                w_t = wapool.tile([P, CHUNK * S], F32, tag="w")
                a_t = wapool.tile([P, CHUNK * S], F32, tag="a")
                nc.sync.dma_start(
                    out=w_t.rearrange("p (f c) -> p f c", c=S),
                    in_=w_d[r0:r0 + CHUNK * P, :].rearrange("(f p) c -> p f c", p=P),
                )
                nc.sync.dma_start(
                    out=a_t.rearrange("p (f c) -> p f c", c=S),
                    in_=a_d[r0:r0 + CHUNK * P, :].rearrange("(f p) c -> p f c", p=P),
                )
                nc.sync.dma_start(
                    out=hebb_all[:, csl].rearrange("p (f c) -> p f c", c=S),
                    in_=h_d[r0:r0 + CHUNK * P, :].rearrange("(f p) c -> p f c", p=P),
                )
                # mw = alpha*hebb + w on DVE: keeps every matmul operand
                # DVE-produced (single sync-wait) and halves the matmul count.
                mw_t = mpool.tile([P, CHUNK * S], F32, tag="mw")
                nc.vector.tensor_mul(out=mw_t[:], in0=a_t[:], in1=hebb_all[:, csl])
                nc.vector.tensor_add(out=mw_t[:], in0=mw_t[:], in1=w_t[:])
                for f in range(CHUNK):
                    t = c * CHUNK + f
                    fsl = slice(f * S, (f + 1) * S)
                    nc.tensor.matmul(
                        z_ps[:], yin_col[:, t:t + 1], mw_t[:, fsl],
                        start=(t == 0), stop=(t == T - 1),
                    )

            # ---- yout = tanh(z + input) ----
            z_sb = cpool.tile([1, S], F32, tag="z_sb")
            nc.vector.tensor_add(out=z_sb[:], in0=z_ps[:], in1=inp_sb[:])
            yout_sb = cpool.tile([1, S], F32, tag="yout")
            nc.scalar.activation(
                yout_sb[:], z_sb[:], mybir.ActivationFunctionType.Tanh
            )
            nc.sync.dma_start(out=yout_d[:], in_=yout_sb[:])
            # eta * yout (rhs of the outer-product matmul)
            eyout = cpool.tile([1, S], F32, tag="eyout")
            nc.vector.tensor_scalar(
                out=eyout[:], in0=yout_sb[:], scalar1=eta_sb[0:1, 0:1], scalar2=None,
                op0=mult,
            )

            # ---- phase B: hebb_new = (1-eta)*hebb + outer(yin, eta*yout) ----
            for c in range(NCHUNKS):
                out_t = opool.tile([P, CHUNK * S], F32, tag="out")
                for f in range(CHUNK):
                    t = c * CHUNK + f
                    fsl = slice(f * S, (f + 1) * S)
                    o_ps = pspool.tile([P, S], F32, tag="outer")
                    nc.tensor.matmul(
                        o_ps[:], yin_row[0:1, t * P:(t + 1) * P], eyout[:],
                        start=True, stop=True,
                    )
                    nc.vector.scalar_tensor_tensor(
                        out=out_t[:, fsl],
                        in0=hebb_all[:, t * S:(t + 1) * S],
                        scalar=ome_b[:, 0:1],
                        in1=o_ps[:],
                        op0=mult, op1=add,
                    )
                r0 = c * CHUNK * P
                nc.sync.dma_start(
                    out=hout_d[r0:r0 + CHUNK * P, :].rearrange(
                        "(f p) c -> p f c", p=P
                    ),
                    in_=out_t.rearrange("p (f c) -> p f c", c=S),
                )

    return nc


def kernel(input, yin, hebb, w, alpha, eta):
    global _last_results

    input = np.asarray(input, dtype=np.float32)
    yin = np.asarray(yin, dtype=np.float32)
    hebb = np.asarray(hebb, dtype=np.float32)
    w = np.asarray(w, dtype=np.float32)
    alpha = np.asarray(alpha, dtype=np.float32)
    eta = np.asarray(eta, dtype=np.float32)

    nc = _build()
    nc.finalize()  # Bacc: run reg-alloc + wait-splitting passes

    eta2d = np.ascontiguousarray(eta.reshape(1, 1))
    yin2d = np.ascontiguousarray(yin.reshape(1, N))
    in_maps = []
    for i in range(NCORES):
        sl = slice(i * S, (i + 1) * S)
        in_maps.append({
            "w_sh": np.ascontiguousarray(w[:, sl]),
            "alpha_sh": np.ascontiguousarray(alpha[:, sl]),
            "hebb_sh": np.ascontiguousarray(hebb[:, sl]),
            "yin_full": yin2d,
            "input_sh": np.ascontiguousarray(input.reshape(1, N)[:, sl]),
            "eta_in": eta2d,
        })

    res = run_bass_kernel_spmd(nc, in_maps, core_ids=list(range(NCORES)))
    _last_results = res

    yout = np.concatenate(
        [res.results[i]["yout_sh"] for i in range(NCORES)], axis=1
    )
    hebb_new = np.concatenate(
        [res.results[i]["hebb_out"] for i in range(NCORES)], axis=1
    )
    return yout, hebb_new


# revision 9
# speedup vs baseline: 1.3027x; 1.3027x over previous
"""Trainium2 Bass kernel for the plastic-network step:

    yout     = tanh(yin @ (w + alpha * hebb) + input)        [1, N]
    hebb_new = (1 - eta) * hebb + eta * outer(yin, yout)     [N, N]

Sharding: all NxN matrices (w, alpha, hebb) are split column-wise across
8 NeuronCores (512 columns each).  The matvec contraction runs over all
4096 rows of the local column shard, so every core computes its own
yout columns independently -- no collectives.  hebb is kept resident in
SBUF between the matvec pass and the hebb-update pass so it is read
from HBM exactly once.

Layout: each 1 MiB DMA chunk covers CHUNK*P consecutive shard rows;
partition p holds rows {r0 + CHUNK*p + f}, i.e. CHUNK consecutive DRAM
rows per partition -> 8 KB contiguous per-partition descriptors.  DMAs
alternate between the two HWDGE rings (sync / scalar engines).
"""

import numpy as np

import concourse.bacc as bacc
import concourse.mybir as mybir
from concourse.bass_utils import run_bass_kernel_spmd
from concourse.tile import TileContext

N = 4096          # full matrix dim
NCORES = 8
S = N // NCORES   # columns per core (512)
P = 128           # SBUF partitions
T = N // P        # row tiles (32)
CHUNK = 4         # row tiles per DMA chunk -> 4*128*512*4B = 1 MiB
NCHUNKS = T // CHUNK

F32 = mybir.dt.float32
BF16 = mybir.dt.bfloat16

_last_results = None  # BassKernelResults of the most recent run (for test.py)


def _build() -> bacc.Bacc:
    nc = bacc.Bacc()

    w_d = nc.dram_tensor("w_sh", [N, S], F32, kind="ExternalInput")
    a_d = nc.dram_tensor("alpha_sh", [N, S], F32, kind="ExternalInput")
    h_d = nc.dram_tensor("hebb_sh", [N, S], F32, kind="ExternalInput")
    yin_d = nc.dram_tensor("yin_full", [1, N], F32, kind="ExternalInput")
    inp_d = nc.dram_tensor("input_sh", [1, S], F32, kind="ExternalInput")
    eta_d = nc.dram_tensor("eta_in", [1, 1], F32, kind="ExternalInput")

    yout_d = nc.dram_tensor("yout_sh", [1, S], F32, kind="ExternalOutput")
    hout_d = nc.dram_tensor("hebb_out", [N, S], F32, kind="ExternalOutput")

    mult = mybir.AluOpType.mult
    add = mybir.AluOpType.add
    dma_rings = [nc.sync, nc.scalar]  # two HWDGE rings

    with TileContext(nc) as tc:
        with (
            tc.tile_pool(name="const", bufs=1) as cpool,
            tc.tile_pool(name="hebb", bufs=1) as hpool,
            tc.tile_pool(name="wa", bufs=3) as wapool,
            tc.tile_pool(name="mm", bufs=2) as mpool,
            tc.tile_pool(name="ob", bufs=3) as opool,
            tc.tile_pool(name="ps", bufs=2, space="PSUM") as pspool,
            tc.tile_pool(name="psz", bufs=1, space="PSUM") as pzpool,
        ):
            # ---- small setup tensors ----
            # yin gathered to match the chunk layout:
            # yin_col[p, c, f] = yin[c*(CHUNK*P) + p*CHUNK + f].
            # It passes through a DVE copy so every matmul depends only on
            # the DVE semaphore (walrus allows one sync-wait per Matmult).
            yin_colr = cpool.tile([P, NCHUNKS, CHUNK], F32, tag="yin_colr")
            nc.sync.dma_start(
                out=yin_colr[:],
                in_=yin_d.rearrange("a (c p f) -> (a p) c f", p=P, f=CHUNK),
            )
            yin_col = cpool.tile([P, NCHUNKS, CHUNK], F32, tag="yin_col")
            nc.vector.tensor_copy(out=yin_col[:], in_=yin_colr[:])
            # yin as a bf16 row (K=1 stationary operand for the outer
            # product; error enters hebb_new scaled by eta*yout ~ 1e-5).
            yin_rowr = cpool.tile([1, N], F32, tag="yin_rowr")
            nc.sync.dma_start(out=yin_rowr[:], in_=yin_d[:])
            yin_row_bf = cpool.tile([1, N], BF16, tag="yin_row_bf")
            nc.vector.tensor_copy(out=yin_row_bf[:], in_=yin_rowr[:])
            inp_sb = cpool.tile([1, S], F32, tag="inp")
            nc.sync.dma_start(out=inp_sb[:], in_=inp_d[:])
            eta_sb = cpool.tile([1, 1], F32, tag="eta")
            nc.sync.dma_start(out=eta_sb[:], in_=eta_d[:])

            # Broadcast eta across partitions (0-stride DMA read from DRAM),
            # then compute (1 - eta).
            etab = cpool.tile([P, 1], F32, tag="etab")
            nc.sync.dma_start(
                out=etab.rearrange("p (c o) -> p c o", o=1),
                in_=eta_d[:].partition_broadcast(P),
            )
            ome_b = cpool.tile([P, 1], F32, tag="ome")  # 1 - eta, per partition
            nc.vector.tensor_scalar(
                out=ome_b[:], in0=etab[:], scalar1=-1.0, scalar2=1.0,
                op0=mult, op1=add,
            )

            # hebb shard stays resident in SBUF: 128 x (32*512) f32 = 8 MiB
            hebb_all = hpool.tile([P, T * S], F32, tag="hebb_all")

            # ---- phase A: z = yin @ (w + alpha*hebb) for the local columns ----
            z_ps = pzpool.tile([1, S], F32, tag="z")
            for c in range(NCHUNKS):
                r0 = c * CHUNK * P
                csl = slice(c * CHUNK * S, (c + 1) * CHUNK * S)
                w_t = wapool.tile([P, CHUNK * S], F32, tag="w")
                a_t = wapool.tile([P, CHUNK * S], F32, tag="a")
                # partition p <- DRAM rows r0 + CHUNK*p .. r0 + CHUNK*p+3
                # (8 KB contiguous per partition)
                dma_rings[c % 2].dma_start(
                    out=w_t.rearrange("p (f c) -> p f c", c=S),
                    in_=w_d[r0:r0 + CHUNK * P, :].rearrange("(p f) c -> p f c", p=P),
                )
                dma_rings[(c + 1) % 2].dma_start(
                    out=a_t.rearrange("p (f c) -> p f c", c=S),
                    in_=a_d[r0:r0 + CHUNK * P, :].rearrange("(p f) c -> p f c", p=P),
                )
                dma_rings[c % 2].dma_start(
                    out=hebb_all[:, csl].rearrange("p (f c) -> p f c", c=S),
                    in_=h_d[r0:r0 + CHUNK * P, :].rearrange("(p f) c -> p f c", p=P),
                )
                # mw = alpha*hebb + w on DVE: keeps every matmul operand
                # DVE-produced (single sync-wait) and halves the matmul count.
                mw_t = mpool.tile([P, CHUNK * S], F32, tag="mw")
                nc.vector.tensor_mul(out=mw_t[:], in0=a_t[:], in1=hebb_all[:, csl])
                nc.vector.tensor_add(out=mw_t[:], in0=mw_t[:], in1=w_t[:])
                for f in range(CHUNK):
                    t = c * CHUNK + f
                    fsl = slice(f * S, (f + 1) * S)
                    nc.tensor.matmul(
                        z_ps[:], yin_col[:, c, f:f + 1], mw_t[:, fsl],
                        start=(t == 0), stop=(t == T - 1),
                    )

            # ---- yout = tanh(z + input) ----
            z_sb = cpool.tile([1, S], F32, tag="z_sb")
            nc.vector.tensor_add(out=z_sb[:], in0=z_ps[:], in1=inp_sb[:])
            yout_sb = cpool.tile([1, S], F32, tag="yout")
            nc.scalar.activation(
                yout_sb[:], z_sb[:], mybir.ActivationFunctionType.Tanh
            )
            nc.sync.dma_start(out=yout_d[:], in_=yout_sb[:])
            # eta * yout in bf16 (rhs of the outer-product matmul)
            eyout = cpool.tile([1, S], F32, tag="eyout")
            nc.vector.tensor_scalar(
                out=eyout[:], in0=yout_sb[:], scalar1=eta_sb[0:1, 0:1], scalar2=None,
                op0=mult,
            )
            eyout_bf = cpool.tile([1, S], BF16, tag="eyout_bf")
            nc.vector.tensor_copy(out=eyout_bf[:], in_=eyout[:])

            # yin_row_bf viewed so that [0, c, f, m] = yin[c*512 + 4m + f]
            yin_lhs = yin_row_bf.rearrange("a (c m f) -> a c f m", m=P, f=CHUNK)

            # ---- phase B: hebb_new = (1-eta)*hebb + outer(yin, eta*yout) ----
            for c in range(NCHUNKS):
                out_t = opool.tile([P, CHUNK * S], F32, tag="out")
                for f in range(CHUNK):
                    t = c * CHUNK + f
                    fsl = slice(f * S, (f + 1) * S)
                    o_ps = pspool.tile([P, S], F32, tag="outer")
                    nc.tensor.matmul(
                        o_ps[:], yin_lhs[0:1, c, f, :], eyout_bf[:],
                        start=True, stop=True,
                    )
                    nc.vector.scalar_tensor_tensor(
                        out=out_t[:, fsl],
                        in0=hebb_all[:, t * S:(t + 1) * S],
                        scalar=ome_b[:, 0:1],
                        in1=o_ps[:],
                        op0=mult, op1=add,
                    )
                r0 = c * CHUNK * P
                dma_rings[c % 2].dma_start(
                    out=hout_d[r0:r0 + CHUNK * P, :].rearrange(
                        "(p f) c -> p f c", p=P
                    ),
                    in_=out_t.rearrange("p (f c) -> p f c", c=S),
                )

    return nc


def kernel(input, yin, hebb, w, alpha, eta):
    global _last_results

    input = np.asarray(input, dtype=np.float32)
    yin = np.asarray(yin, dtype=np.float32)
    hebb = np.asarray(hebb, dtype=np.float32)
    w = np.asarray(w, dtype=np.float32)
    alpha = np.asarray(alpha, dtype=np.float32)
    eta = np.asarray(eta, dtype=np.float32)

    nc = _build()
    nc.finalize()  # Bacc: run reg-alloc + wait-splitting passes

    eta2d = np.ascontiguousarray(eta.reshape(1, 1))
    yin2d = np.ascontiguousarray(yin.reshape(1, N))
    in_maps = []
    for i in range(NCORES):
        sl = slice(i * S, (i + 1) * S)
        in_maps.append({
            "w_sh": np.ascontiguousarray(w[:, sl]),
            "alpha_sh": np.ascontiguousarray(alpha[:, sl]),
            "hebb_sh": np.ascontiguousarray(hebb[:, sl]),
            "yin_full": yin2d,
            "input_sh": np.ascontiguousarray(input.reshape(1, N)[:, sl]),
            "eta_in": eta2d,
        })

    res = run_bass_kernel_spmd(nc, in_maps, core_ids=list(range(NCORES)))
    _last_results = res

    yout = np.concatenate(
        [res.results[i]["yout_sh"] for i in range(NCORES)], axis=1
    )
    hebb_new = np.concatenate(
        [res.results[i]["hebb_out"] for i in range(NCORES)], axis=1
    )
    return yout, hebb_new


# revision 12
# speedup vs baseline: 1.3874x; 1.0651x over previous
"""Trainium2 Bass kernel for the plastic-network step:

    yout     = tanh(yin @ (w + alpha * hebb) + input)        [1, N]
    hebb_new = (1 - eta) * hebb + eta * outer(yin, yout)     [N, N]

Sharding: all NxN matrices (w, alpha, hebb) are split column-wise across
8 NeuronCores (512 columns each).  The matvec contraction runs over all
4096 rows of the local column shard, so every core computes its own
yout columns independently -- no collectives.  hebb is kept resident in
SBUF between the matvec pass and the hebb-update pass so it is read
from HBM exactly once.

Layout: each DMA chunk covers nt*P consecutive shard rows; partition p
holds rows {r0 + nt*p + f}, i.e. nt consecutive DRAM rows per partition
-> 8-16 KB contiguous per-partition descriptors.  DMAs alternate
between the two HWDGE rings (sync / scalar engines).  The matching yin
permutations (column layout for the matvec lhsT, row layout for the
outer-product lhsT) and the eta scalars are precomputed on the host.
"""

import numpy as np

import concourse.bacc as bacc
import concourse.mybir as mybir
from concourse.bass_utils import run_bass_kernel_spmd
from concourse.tile import TileContext

N = 4096          # full matrix dim
NCORES = 8
S = N // NCORES   # columns per core (512)
P = 128           # SBUF partitions
T = N // P        # row tiles (32)
CHUNK = 4         # max row tiles per DMA chunk -> 1 MiB
# DMA chunk plan (start_tile, ntiles): big chunks, tapered tail so the
# final DMA->DVE->matmul chain exposes less serial latency.
CHUNKS = [(0, 4), (4, 4), (8, 4), (12, 4), (16, 4), (20, 4), (24, 4),
          (28, 2), (30, 2)]
GRP = 2           # tiles per DVE op / per PSUM group in phase B

F32 = mybir.dt.float32
BF16 = mybir.dt.bfloat16

AUX_W = 36        # aux packed tensor: [0:32]=yin_col, 32=1-eta, 33=eta

_last_results = None  # BassKernelResults of the most recent run (for test.py)


def _row_index_map() -> np.ndarray:
    """idx[t, p] = shard row held by (tile t, partition p) under CHUNKS."""
    idx = np.zeros((T, P), dtype=np.int64)
    for t0, nt in CHUNKS:
        for f in range(nt):
            idx[t0 + f, :] = t0 * P + nt * np.arange(P) + f
    return idx


def _build() -> bacc.Bacc:
    nc = bacc.Bacc()

    w_d = nc.dram_tensor("w_sh", [N, S], F32, kind="ExternalInput")
    a_d = nc.dram_tensor("alpha_sh", [N, S], F32, kind="ExternalInput")
    h_d = nc.dram_tensor("hebb_sh", [N, S], F32, kind="ExternalInput")
    ylhs_d = nc.dram_tensor("yin_lhs", [1, N], F32, kind="ExternalInput")
    inp_d = nc.dram_tensor("input_sh", [1, S], F32, kind="ExternalInput")
    aux_d = nc.dram_tensor("aux_in", [P, AUX_W], F32, kind="ExternalInput")

    yout_d = nc.dram_tensor("yout_sh", [1, S], F32, kind="ExternalOutput")
    hout_d = nc.dram_tensor("hebb_out", [N, S], F32, kind="ExternalOutput")

    mult = mybir.AluOpType.mult
    add = mybir.AluOpType.add
    rings = [nc.sync, nc.scalar]  # two HWDGE rings

    with TileContext(nc) as tc:
        with (
            tc.tile_pool(name="const", bufs=1) as cpool,
            tc.tile_pool(name="hebb", bufs=1) as hpool,
            tc.tile_pool(name="wa", bufs=3) as wapool,
            tc.tile_pool(name="mm", bufs=2) as mpool,
            tc.tile_pool(name="ob", bufs=3) as opool,
            tc.tile_pool(name="ps", bufs=2, space="PSUM") as pspool,
            tc.tile_pool(name="psz", bufs=1, space="PSUM") as pzpool,
        ):
            # ---- small setup tensors ----
            aux_sb = cpool.tile([P, AUX_W], F32, tag="aux")
            nc.scalar.dma_start(out=aux_sb[:], in_=aux_d[:])
            inp_sb = cpool.tile([1, S], F32, tag="inp")
            nc.scalar.dma_start(out=inp_sb[:], in_=inp_d[:])
            ylhsr = cpool.tile([1, N], F32, tag="ylhsr")
            nc.sync.dma_start(out=ylhsr[:], in_=ylhs_d[:])

            # yin_col[p, t] = yin[idx[t, p]] -- precomputed on host in aux.
            # The DVE copy makes every matmul operand DVE-produced (walrus
            # allows one sync-wait per Matmult).
            yin_col = cpool.tile([P, T], F32, tag="yin_col")
            nc.vector.tensor_copy(out=yin_col[:], in_=aux_sb[:, 0:T])
            # ylhs_bf[0, t*P + m] = yin[idx[t, m]] in bf16 (K=1 stationary
            # operand of the outer product; error enters hebb_new scaled by
            # eta*yout ~ 1e-5).
            ylhs_bf = cpool.tile([1, N], BF16, tag="ylhs_bf")
            nc.vector.tensor_copy(out=ylhs_bf[:], in_=ylhsr[:])

            ome_b = aux_sb[:, 32:33]        # 1 - eta, per partition
            eta_s = aux_sb[0:1, 33:34]      # eta as a [1,1] scalar

            # hebb shard stays resident in SBUF: 128 x (32*512) f32 = 8 MiB
            hebb_all = hpool.tile([P, T * S], F32, tag="hebb_all")

            # ---- phase A: z = yin @ (w + alpha*hebb) for the local columns ----
            z_ps = pzpool.tile([1, S], F32, tag="z")
            for ci, (t0, nt) in enumerate(CHUNKS):
                r0 = t0 * P
                csl = slice(t0 * S, (t0 + nt) * S)
                w_t = wapool.tile([P, CHUNK * S], F32, tag="w", name="w_t")[:, :nt * S]
                a_t = wapool.tile([P, CHUNK * S], F32, tag="a", name="a_t")[:, :nt * S]
                # partition p <- DRAM rows r0 + nt*p .. r0 + nt*p + nt-1
                # (nt*2 KB contiguous per partition)
                rings[ci % 2].dma_start(
                    out=w_t.rearrange("p (f c) -> p f c", c=S),
                    in_=w_d[r0:r0 + nt * P, :].rearrange("(p f) c -> p f c", p=P),
                )
                rings[(ci + 1) % 2].dma_start(
                    out=a_t.rearrange("p (f c) -> p f c", c=S),
                    in_=a_d[r0:r0 + nt * P, :].rearrange("(p f) c -> p f c", p=P),
                )
                rings[ci % 2].dma_start(
                    out=hebb_all[:, csl].rearrange("p (f c) -> p f c", c=S),
                    in_=h_d[r0:r0 + nt * P, :].rearrange("(p f) c -> p f c", p=P),
                )
                # mw = alpha*hebb + w on DVE in GRP-tile slices: keeps every
                # matmul operand DVE-produced and pipelines finer.
                mw_t = mpool.tile([P, CHUNK * S], F32, tag="mw", name="mw_t")[:, :nt * S]
                for g0 in range(0, nt, GRP):
                    gsl = slice(g0 * S, (g0 + GRP) * S)
                    hsl = slice((t0 + g0) * S, (t0 + g0 + GRP) * S)
                    nc.vector.tensor_mul(
                        out=mw_t[:, gsl], in0=a_t[:, gsl], in1=hebb_all[:, hsl]
                    )
                    nc.vector.tensor_add(
                        out=mw_t[:, gsl], in0=mw_t[:, gsl], in1=w_t[:, gsl]
                    )
                    for f in range(g0, g0 + GRP):
                        t = t0 + f
                        nc.tensor.matmul(
                            z_ps[:], yin_col[:, t:t + 1],
                            mw_t[:, f * S:(f + 1) * S],
                            start=(t == 0), stop=(t == T - 1),
                        )

            # ---- yout = tanh(z + input) ----
            z_sb = cpool.tile([1, S], F32, tag="z_sb")
            nc.vector.tensor_add(out=z_sb[:], in0=z_ps[:], in1=inp_sb[:])
            yout_sb = cpool.tile([1, S], F32, tag="yout")
            nc.scalar.activation(
                yout_sb[:], z_sb[:], mybir.ActivationFunctionType.Tanh
            )
            nc.sync.dma_start(out=yout_d[:], in_=yout_sb[:])
            # eta * yout in bf16 (rhs of the outer-product matmul)
            eyout = cpool.tile([1, S], F32, tag="eyout")
            nc.vector.tensor_scalar(
                out=eyout[:], in0=yout_sb[:], scalar1=eta_s, scalar2=None,
                op0=mult,
            )
            eyout_bf = cpool.tile([1, S], BF16, tag="eyout_bf")
            nc.vector.tensor_copy(out=eyout_bf[:], in_=eyout[:])

            # ---- phase B: hebb_new = (1-eta)*hebb + outer(yin, eta*yout) ----
            # GRP-tile PSUM groups (2 banks) so one scalar_tensor_tensor
            # covers GRP*512 columns.
            for ci, (t0, nt) in enumerate(CHUNKS):
                out_t = opool.tile([P, CHUNK * S], F32, tag="out", name="out_t")[:, :nt * S]
                for g0 in range(0, nt, GRP):
                    o_ps = pspool.tile([P, GRP * S], F32, tag="outer")
                    for j in range(GRP):
                        t = t0 + g0 + j
                        nc.tensor.matmul(
                            o_ps[:, j * S:(j + 1) * S],
                            ylhs_bf[0:1, t * P:(t + 1) * P],
                            eyout_bf[:],
                            start=True, stop=True,
                        )
                    nc.vector.scalar_tensor_tensor(
                        out=out_t[:, g0 * S:(g0 + GRP) * S],
                        in0=hebb_all[:, (t0 + g0) * S:(t0 + g0 + GRP) * S],
                        scalar=ome_b,
                        in1=o_ps[:],
                        op0=mult, op1=add,
                    )
                r0 = t0 * P
                rings[ci % 2].dma_start(
                    out=hout_d[r0:r0 + nt * P, :].rearrange(
                        "(p f) c -> p f c", p=P
                    ),
                    in_=out_t.rearrange("p (f c) -> p f c", c=S),
                )

    return nc


def kernel(input, yin, hebb, w, alpha, eta):
    global _last_results

    input = np.asarray(input, dtype=np.float32)
    yin = np.asarray(yin, dtype=np.float32)
    hebb = np.asarray(hebb, dtype=np.float32)
    w = np.asarray(w, dtype=np.float32)
    alpha = np.asarray(alpha, dtype=np.float32)
    eta = np.asarray(eta, dtype=np.float32)

    nc = _build()
    nc.finalize()  # Bacc: run reg-alloc + wait-splitting passes

    yin_flat = yin.reshape(N)
    eta_v = float(eta.reshape(-1)[0])
    idx = _row_index_map()
    ycols = yin_flat[idx]                       # [T, P]
    # aux: [0:32] = yin in tile-column layout, 32 = 1-eta, 33 = eta
    aux = np.zeros((P, AUX_W), dtype=np.float32)
    aux[:, 0:T] = ycols.T
    aux[:, 32] = 1.0 - eta_v
    aux[:, 33] = eta_v
    aux = np.ascontiguousarray(aux)
    ylhs = np.ascontiguousarray(ycols.reshape(1, N))

    in_maps = []
    for i in range(NCORES):
        sl = slice(i * S, (i + 1) * S)
        in_maps.append({
            "w_sh": np.ascontiguousarray(w[:, sl]),
            "alpha_sh": np.ascontiguousarray(alpha[:, sl]),
            "hebb_sh": np.ascontiguousarray(hebb[:, sl]),
            "yin_lhs": ylhs,
            "input_sh": np.ascontiguousarray(input.reshape(1, N)[:, sl]),
            "aux_in": aux,
        })

    res = run_bass_kernel_spmd(nc, in_maps, core_ids=list(range(NCORES)))
    _last_results = res

    yout = np.concatenate(
        [res.results[i]["yout_sh"] for i in range(NCORES)], axis=1
    )
    hebb_new = np.concatenate(
        [res.results[i]["hebb_out"] for i in range(NCORES)], axis=1
    )
    return yout, hebb_new
